# revision 12
# baseline (speedup 1.0000x reference)
"""LocalRNN (windowed LSTM) Trainium2 kernel.

Problem: x (8, 2048, 128); for every position s, run a W=16-step LSTM over
x[b, s-15 .. s] (zero-padded) with h0=c0=0; output the final hidden state.

Sharding: batch across the 8 cores (core c handles batch c; windows never
cross batches, so no halo is needed).

Layout is feature-major: hidden dim d=128 on SBUF partitions, positions on
the free dim.  x is transposed/padded host-side to xT (128, 15+2048+1), and
the output comes back as hT (128, 2048), transposed on host.  Per step and
512-position chunk:

  psum[d, 4*512] = whh_j @ h  (+)  I @ xg_j_slice     (fp32r matmuls, PSUM acc)
  s  = sigmoid(psum)                 (ONE ACT pass across all 4 gate banks)
  u  = (s_g - 0.5) * s_i             (DVE fused scalar_tensor_tensor)
  t2 = s_f * c                       (GPSIMD tensor_tensor)
  c  = 2*u + t2                      (DVE fused)
  tc = tanh(c)                       (ACT, same table set as sigmoid)
  h  = tc * s_o                      (DVE or GPSIMD tensor_tensor)

The gate tanh is sigmoid-ized (tanh(g) = 2*sigmoid(2g) - 1, the *2 folded
into host-pre-scaled g-gate rows of the weights) so the gate pass is a
single wide sigmoid; the cell tanh stays a real tanh so h needs no
post-scaling.  xg = w_ih @ x + (b_ih + b_hh) is precomputed per 512-column
segment, interleaved with step-0 chunks (which read xT directly with
per-gate bias sigmoids so nothing waits on xg).

Host path: the compiled NEFF, the jitted 8-core shard_map executable and
the device-resident weight buffers are all built once per process and
cached; each kernel() call only uploads x, executes, and fetches y.
"""

import numpy as np

import concourse.mybir as mybir
import concourse.tile as tile
from concourse import bacc

B, S, D = 8, 2048, 128
H4 = 4 * D
W = 16
PAD = W - 1              # 15 zero-padded positions in front
CH = 512                 # positions per chunk (= one fp32 PSUM bank)
NCH = S // CH            # 4
XW = PAD + S + 1         # padded xT width (2064, kept even)

F32 = mybir.dt.float32
F32R = mybir.dt.float32r
BF16 = mybir.dt.bfloat16
SIG = mybir.ActivationFunctionType.Sigmoid
TANH = mybir.ActivationFunctionType.Tanh
ADD = mybir.AluOpType.add
MUL = mybir.AluOpType.mult


def build_nc(mm_dtype=F32R, reps=1, h_gpsimd=(0, 1, 2, 3), warm_table=True,
             group_mm=False, step0_direct=True, whh_bf16=False, xg_bf16=False,
             x_bf16=False, y_bf16=False,
             early_order="c0,s0,c1,s1,c2,s2,c3,s3,s4"):
    nc = bacc.Bacc("TRN2")
    x_dt = BF16 if x_bf16 else F32R
    x_d = nc.dram_tensor("xT", (D, XW), x_dt, kind="ExternalInput")
    wih_dt = BF16 if x_bf16 else F32R
    wih_d = nc.dram_tensor("wihT", (D, H4),
                           BF16 if x_bf16 else F32, kind="ExternalInput")
    whh_dt = BF16 if whh_bf16 else F32R
    whh_d = nc.dram_tensor("whhT", (D, H4),
                           BF16 if whh_bf16 else F32, kind="ExternalInput")
    b_d = nc.dram_tensor("bcols", (D, 4), F32, kind="ExternalInput")
    id_dt = BF16 if xg_bf16 else F32R
    id_d = nc.dram_tensor("ident", (D, D), id_dt, kind="ExternalInput")
    y_dt = BF16 if y_bf16 else F32
    y_d = nc.dram_tensor("y", (D, S), y_dt, kind="ExternalOutput")

    with tile.TileContext(nc) as tc:
        with (
            tc.tile_pool(name="const", bufs=1) as cpool,
            tc.tile_pool(name="persist", bufs=1) as ppool,
            tc.tile_pool(name="state", bufs=1) as hpool,
            tc.tile_pool(name="work", bufs=3) as wpool,
        ):
            wih = cpool.tile([D, H4], wih_dt, name="wih")
            whh = cpool.tile([D, H4], whh_dt, name="whh")
            bc = cpool.tile([D, 4], F32, name="bc")
            ident = cpool.tile([D, D], id_dt, name="ident")
            xT = ppool.tile([D, XW], x_dt, name="xT")
            QW = XW // 4  # 516

            if warm_table:
                z16 = cpool.tile([D, 16], F32, name="z16")
                zs = cpool.tile([D, 16], F32, name="zs")
                nc.vector.memset(z16, 0.0)
                nc.scalar.activation(zs, z16, SIG)

            # DMA order matters: the first step-0 chunk needs xT q0 + wih +
            # bc; everything else can land later.
            nc.sync.dma_start(out=xT[:, 0:QW], in_=x_d.ap()[:, 0:QW])
            nc.sync.dma_start(
                out=wih,
                in_=wih_d.ap() if x_bf16 else wih_d.ap().bitcast(F32R),
            )
            nc.sync.dma_start(out=bc, in_=b_d.ap())
            for q in range(1, 4):
                nc.sync.dma_start(
                    out=xT[:, q * QW : (q + 1) * QW],
                    in_=x_d.ap()[:, q * QW : (q + 1) * QW],
                )
            nc.sync.dma_start(
                out=whh,
                in_=whh_d.ap() if whh_bf16 else whh_d.ap().bitcast(F32R),
            )
            nc.sync.dma_start(out=ident, in_=id_d.ap())
            xg_dt = BF16 if xg_bf16 else F32R
            xg = [ppool.tile([D, XW], xg_dt, name=f"xg{j}") for j in range(4)]

            h = [hpool.tile([D, CH], F32R, name=f"h{k}") for k in range(NCH)]
            c = [hpool.tile([D, CH], F32, name=f"c{k}") for k in range(NCH)]

            sig_insts = []
            hwr_insts = []
            segs = [(k * CH, CH) for k in range(4)] + [(4 * CH, XW - 4 * CH)]

            with tc.tile_pool(name="psum_g", bufs=2, space="PSUM") as pgp:

                def new_pg():
                    return pgp.tile([D, 4 * CH], F32, name="pg", tag="pg")

                def emit_xg_seg(si):
                    off, ln = segs[si]
                    pg = new_pg()
                    for j in range(4):
                        bank = pg[:, j * CH : j * CH + ln]
                        nc.tensor.matmul(
                            bank,
                            wih[:, j * D : (j + 1) * D],
                            xT[:, off : off + ln],
                            start=True,
                            stop=True,
                        )
                        nc.vector.tensor_scalar_add(
                            out=xg[j][:, off : off + ln],
                            in0=bank,
                            scalar1=bc[:, j : j + 1],
                        )

                def emit_cell_tail(w, k, s):
                    s_i = s[:, 0:CH]
                    s_f = s[:, CH : 2 * CH]
                    s_o = s[:, 2 * CH : 3 * CH]
                    s_g = s[:, 3 * CH : 4 * CH]
                    u = wpool.tile([D, CH], F32, name="u", tag="u")
                    nc.vector.scalar_tensor_tensor(u, s_g, -0.5, s_i, ADD, MUL)
                    if w > 0:
                        t2 = wpool.tile([D, CH], F32, name="t2", tag="t2")
                        nc.vector.tensor_tensor(t2, s_f, c[k], MUL)
                        nc.vector.scalar_tensor_tensor(c[k], u, 2.0, t2, MUL, ADD)
                    else:
                        nc.vector.tensor_scalar_mul(c[k], u, 2.0)
                    tc_t = wpool.tile([D, CH], F32, name="tc", tag="tc")
                    nc.scalar.activation(tc_t, c[k], TANH)
                    h_eng = nc.gpsimd if k in h_gpsimd else nc.vector
                    hwr_insts.append(
                        h_eng.tensor_tensor(h[k], tc_t, s_o, MUL)
                    )

                def emit_step0_chunk(k):
                    pg = new_pg()
                    s = wpool.tile([D, 4 * CH], F32, name="s", tag="s")
                    if step0_direct:
                        for j in range(4):
                            nc.tensor.matmul(
                                pg[:, j * CH : (j + 1) * CH],
                                wih[:, j * D : (j + 1) * D],
                                xT[:, k * CH : (k + 1) * CH],
                                start=True,
                                stop=True,
                            )
                        for j in range(4):
                            sig_insts.append(
                                nc.scalar.activation(
                                    s[:, j * CH : (j + 1) * CH],
                                    pg[:, j * CH : (j + 1) * CH],
                                    SIG,
                                    bias=bc[:, j : j + 1],
                                )
                            )
                    else:
                        for j in range(4):
                            nc.tensor.matmul(
                                pg[:, j * CH : (j + 1) * CH],
                                ident,
                                xg[j][:, k * CH : (k + 1) * CH],
                                start=True,
                                stop=True,
                            )
                        sig_insts.append(nc.scalar.activation(s, pg, SIG))
                    emit_cell_tail(0, k, s)

                def emit_step_chunk(w, k):
                    pg = new_pg()
                    if group_mm:
                        for j in range(4):
                            nc.tensor.matmul(
                                pg[:, j * CH : (j + 1) * CH],
                                whh[:, j * D : (j + 1) * D],
                                h[k],
                                start=True,
                                stop=True,
                            )
                        for j in range(4):
                            xsl = xg[j][:, k * CH + w : k * CH + w + CH]
                            nc.tensor.matmul(
                                pg[:, j * CH : (j + 1) * CH],
                                ident,
                                xsl,
                                start=False,
                                stop=True,
                                skip_group_check=True,
                            )
                    else:
                        for j in range(4):
                            bank = pg[:, j * CH : (j + 1) * CH]
                            xsl = xg[j][:, k * CH + w : k * CH + w + CH]
                            nc.tensor.matmul(
                                bank,
                                whh[:, j * D : (j + 1) * D],
                                h[k],
                                start=True,
                                stop=False,
                            )
                            nc.tensor.matmul(
                                bank, ident, xsl, start=False, stop=True
                            )
                    s = wpool.tile([D, 4 * CH], F32, name="s", tag="s")
                    sig_insts.append(nc.scalar.activation(s, pg, SIG))
                    emit_cell_tail(w, k, s)

                total_w = [wi for _ in range(reps) for wi in range(W)]
                for tok in early_order.split(","):
                    if tok.startswith("c"):
                        emit_step0_chunk(int(tok[1:]))
                    else:
                        emit_xg_seg(int(tok[1:]))
                for wi, w in enumerate(total_w):
                    if wi == 0:
                        continue
                    for k in range(NCH):
                        emit_step_chunk(w, k)

            # output: h chunks straight to DRAM (host transposes back)
            if y_bf16:
                yb = [hpool.tile([D, CH], BF16, name=f"yb{k}")
                      for k in range(NCH)]
                for k in range(NCH):
                    nc.gpsimd.tensor_copy(yb[k], h[k].bitcast(F32))
                    nc.sync.dma_start(
                        out=y_d.ap()[:, k * CH : (k + 1) * CH], in_=yb[k]
                    )
            else:
                for k in range(NCH):
                    nc.sync.dma_start(
                        out=y_d.ap()[:, k * CH : (k + 1) * CH],
                        in_=h[k].bitcast(F32),
                    )
    nc.compile()
    return nc


def prep_weights(w_ih, w_hh, b_ih, b_hh):
    """Gate-reorder to [i, f, o, g], fold both biases together, pre-scale the
    g-gate rows by 2 (its tanh is computed as 2*sigmoid(2g) - 1)."""
    w_ih = np.asarray(w_ih, np.float32)
    w_hh = np.asarray(w_hh, np.float32)
    b = np.asarray(b_ih, np.float32) + np.asarray(b_hh, np.float32)
    perm = np.r_[0:128, 128:256, 384:512, 256:384]
    sc = np.repeat(np.float32([1, 1, 1, 2]), D)
    wihT = np.ascontiguousarray((w_ih[perm] * sc[:, None]).T, np.float32)
    whhT = np.ascontiguousarray((w_hh[perm] * sc[:, None]).T, np.float32)
    bcols = np.ascontiguousarray((b[perm] * sc).reshape(4, D).T, np.float32)
    return wihT, whhT, bcols


def prep_x(x):
    """(B, S, D) -> per-core padded transposed xT (B, D, PAD+S+1)."""
    x = np.asarray(x, np.float32)
    xt = np.zeros((B, D, XW), np.float32)
    xt[:, :, PAD : PAD + S] = x.transpose(0, 2, 1)
    return xt


class _Runner:
    """Process-lifetime cache: compiled NEFF + jitted 8-core executable +
    device-resident weights.  Per call: upload x, execute, fetch y."""

    BUILD_KWARGS = {"x_bf16": True, "y_bf16": True}

    def __init__(self, build_kwargs=None):
        import jax
        from jax.sharding import Mesh, PartitionSpec, NamedSharding
        from jax.experimental.shard_map import shard_map
        from concourse import bass2jax as b2j
        import ml_dtypes

        self._jax = jax
        self._bf16 = ml_dtypes.bfloat16
        b2j.install_neuronx_cc_hook()
        if build_kwargs is None:
            build_kwargs = dict(self.BUILD_KWARGS)
        self.build_kwargs = build_kwargs
        self._x_bf16 = bool(build_kwargs.get("x_bf16", False))
        self._y_bf16 = bool(build_kwargs.get("y_bf16", False))
        self.nc = build_nc(**build_kwargs)
        nc = self.nc
        partition_name = (
            nc.partition_id_tensor.name if nc.partition_id_tensor else None
        )
        in_names, out_names, out_avals, zero_outs = [], [], [], []
        for alloc in nc.m.functions[0].allocations:
            if not isinstance(alloc, mybir.MemoryLocationSet):
                continue
            name = alloc.memorylocations[0].name
            if alloc.kind == "ExternalInput":
                if name != partition_name:
                    in_names.append(name)
            elif alloc.kind == "ExternalOutput":
                shape = tuple(alloc.tensor_shape)
                dtype = mybir.dt.np(alloc.dtype)
                out_names.append(name)
                out_avals.append(jax.core.ShapedArray(shape, dtype))
                zero_outs.append(np.zeros(shape, dtype))
        self.in_names = in_names
        self.out_names = out_names
        all_in_names = list(in_names) + out_names
        if partition_name is not None:
            all_in_names.append(partition_name)

        def _body(*args):
            operands = list(args)
            if partition_name is not None:
                operands.append(b2j.partition_id_tensor())
            outs = b2j._bass_exec_p.bind(
                *operands,
                out_avals=tuple(out_avals),
                in_names=tuple(all_in_names),
                out_names=tuple(out_names),
                lowering_input_output_aliases=(),
                sim_require_finite=True,
                sim_require_nnan=True,
                nc=nc,
            )
            return tuple(outs)

        devices = jax.devices()[:B]
        mesh = Mesh(np.asarray(devices), ("core",))
        n_params = len(in_names)
        n_outs = len(out_names)
        self.sharded = jax.jit(
            shard_map(
                _body,
                mesh=mesh,
                in_specs=(PartitionSpec("core"),) * (n_params + n_outs),
                out_specs=(PartitionSpec("core"),) * n_outs,
                check_rep=False,
            ),
            keep_unused=True,
        )
        self.sharding = NamedSharding(mesh, PartitionSpec("core"))
        self.dev_zeros = [
            jax.device_put(
                np.zeros((B * z.shape[0], *z.shape[1:]), z.dtype), self.sharding
            )
            for z in zero_outs
        ]
        self._wkey = None
        self._dev_w = None
        # reusable host staging buffer for the concatenated xT
        xdt = self._bf16 if self._x_bf16 else np.float32
        self._xbuf = np.zeros((B * D, XW), xdt)

    def _stage_weights(self, w_ih, w_hh, b_ih, b_hh):
        w_ih = np.asarray(w_ih, np.float32)
        w_hh = np.asarray(w_hh, np.float32)
        b_ih = np.asarray(b_ih, np.float32)
        b_hh = np.asarray(b_hh, np.float32)
        key = (
            w_ih.tobytes(), w_hh.tobytes(), b_ih.tobytes(), b_hh.tobytes(),
        )
        if self._wkey == key:
            return
        wihT, whhT, bcols = prep_weights(w_ih, w_hh, b_ih, b_hh)
        if self._x_bf16:
            wihT = wihT.astype(self._bf16)
        ident = np.eye(D, dtype=np.float32)
        per_name = {"wihT": wihT, "whhT": whhT, "bcols": bcols, "ident": ident}
        self._dev_w = {
            nm: self._jax.device_put(
                np.concatenate([arr] * B, 0), self.sharding
            )
            for nm, arr in per_name.items()
        }
        self._wkey = key

    def __call__(self, x, w_ih, w_hh, b_ih, b_hh):
        self._stage_weights(w_ih, w_hh, b_ih, b_hh)
        x = np.asarray(x, np.float32)
        xb = self._xbuf
        for bidx in range(B):
            xb[bidx * D : (bidx + 1) * D, PAD : PAD + S] = x[bidx].T
        args = [
            xb if nm == "xT" else self._dev_w[nm] for nm in self.in_names
        ]
        out = self.sharded(*args, *self.dev_zeros)
        y = np.asarray(out[0])                      # (B*D, S) f32 or bf16
        res = np.empty((B, S, D), np.float32)
        yr = y.reshape(B, D, S)
        for bidx in range(B):
            res[bidx] = yr[bidx].T                  # casts bf16 -> f32
        return res


_RUNNER = None


def _get_runner():
    global _RUNNER
    if _RUNNER is None:
        _RUNNER = _Runner()
    return _RUNNER


def kernel(x, w_ih, w_hh, b_ih, b_hh, window_size):
    assert int(window_size) == W, window_size
    return _get_runner()(x, w_ih, w_hh, b_ih, b_hh)


# ---- legacy helpers kept for test harnesses ---------------------------------

_NC_CACHE = {}


def _get_nc(mm_dtype=F32R):
    key = str(mm_dtype)
    if key not in _NC_CACHE:
        _NC_CACHE[key] = build_nc(mm_dtype)
    return _NC_CACHE[key]


def run(x, w_ih, w_hh, b_ih, b_hh, trace=False, mm_dtype=F32R, **spmd_kwargs):
    from concourse.bass_utils import run_bass_kernel_spmd

    x = np.asarray(x, np.float32)
    assert x.shape == (B, S, D), x.shape
    wihT, whhT, bcols = prep_weights(w_ih, w_hh, b_ih, b_hh)
    xt = prep_x(x)
    nc = _get_nc(mm_dtype)
    ident = np.eye(D, dtype=np.float32)
    in_maps = [
        {"xT": xt[cid], "wihT": wihT, "whhT": whhT, "bcols": bcols,
         "ident": ident}
        for cid in range(B)
    ]
    res = run_bass_kernel_spmd(
        nc, in_maps, core_ids=list(range(B)), trace=trace, **spmd_kwargs
    )
    out = np.ascontiguousarray(
        np.stack([res.results[cid]["y"] for cid in range(B)], 0).transpose(
            0, 2, 1
        )
    )
    return out, res


# revision 22
# speedup vs baseline: 1.0193x; 1.0193x over previous
"""LocalRNN (windowed LSTM) Trainium2 kernel.

Problem: x (8, 2048, 128); for every position s, run a W=16-step LSTM over
x[b, s-15 .. s] (zero-padded) with h0=c0=0; output the final hidden state.

Sharding: batch across the 8 cores (core c handles batch c; windows never
cross batches, so no halo is needed).

Layout is feature-major: hidden dim d=128 on SBUF partitions, positions on
the free dim.  x is transposed/padded host-side to xT (128, 15+2048+1), and
the output comes back as hT (128, 2048), transposed on host.  Per step and
512-position chunk:

  psum[d, 4*512] = whh_j @ h  (+)  I @ xg_j_slice     (fp32r matmuls, PSUM acc)
  s  = sigmoid(psum)                 (ONE ACT pass across all 4 gate banks)
  u  = (s_g - 0.5) * s_i             (DVE fused scalar_tensor_tensor)
  t2 = s_f * c                       (GPSIMD tensor_tensor)
  c  = 2*u + t2                      (DVE fused)
  tc = tanh(c)                       (ACT, same table set as sigmoid)
  h  = tc * s_o                      (DVE or GPSIMD tensor_tensor)

The gate tanh is sigmoid-ized (tanh(g) = 2*sigmoid(2g) - 1, the *2 folded
into host-pre-scaled g-gate rows of the weights) so the gate pass is a
single wide sigmoid; the cell tanh stays a real tanh so h needs no
post-scaling.  xg = w_ih @ x + (b_ih + b_hh) is precomputed per 512-column
segment, interleaved with step-0 chunks (which read xT directly with
per-gate bias sigmoids so nothing waits on xg).

Host path: the compiled NEFF, the jitted 8-core shard_map executable and
the device-resident weight buffers are all built once per process and
cached; each kernel() call only uploads x, executes, and fetches y.
"""

import numpy as np

import concourse.mybir as mybir
import concourse.tile as tile
from concourse import bacc

B, S, D = 8, 2048, 128
H4 = 4 * D
W = 16
PAD = W - 1              # 15 zero-padded positions in front
CH = 512                 # positions per chunk (= one fp32 PSUM bank)
NCH = S // CH            # 4
XW = PAD + S + 1         # padded xT width (2064, kept even)

F32 = mybir.dt.float32
F32R = mybir.dt.float32r
BF16 = mybir.dt.bfloat16
SIG = mybir.ActivationFunctionType.Sigmoid
TANH = mybir.ActivationFunctionType.Tanh
ADD = mybir.AluOpType.add
MUL = mybir.AluOpType.mult


def build_nc(mm_dtype=F32R, reps=1, h_gpsimd=(0, 1, 2, 3), warm_table=True,
             group_mm=False, step0_direct=True, whh_bf16=False, xg_bf16=False,
             x_bf16=False, y_bf16=False, io_rows=False,
             io_rows_in=None, io_rows_out=None,
             early_order="c0,s0,c1,s1,c2,s2,c3,s3,s4"):
    if io_rows_in is None:
        io_rows_in = io_rows
    if io_rows_out is None:
        io_rows_out = io_rows
    if io_rows_in or io_rows_out:
        x_bf16 = True
        y_bf16 = True
    nc = bacc.Bacc("TRN2")
    x_dt = BF16 if x_bf16 else F32R
    if io_rows_in:
        # position-major input: device transposes via the DMA XBAR
        x_d = nc.dram_tensor("xR", (S, D), BF16, kind="ExternalInput")
    else:
        x_d = nc.dram_tensor("xT", (D, XW), x_dt, kind="ExternalInput")
    wih_dt = BF16 if x_bf16 else F32R
    wih_d = nc.dram_tensor("wihT", (D, H4),
                           BF16 if x_bf16 else F32, kind="ExternalInput")
    whh_dt = BF16 if whh_bf16 else F32R
    whh_d = nc.dram_tensor("whhT", (D, H4),
                           BF16 if whh_bf16 else F32, kind="ExternalInput")
    b_d = nc.dram_tensor("bcols", (D, 4), F32, kind="ExternalInput")
    id_dt = BF16 if xg_bf16 else F32R
    id_d = nc.dram_tensor("ident", (D, D), id_dt, kind="ExternalInput")
    y_dt = BF16 if y_bf16 else F32
    if io_rows_out:
        y_d = nc.dram_tensor("y", (S, D), BF16, kind="ExternalOutput")
    else:
        y_d = nc.dram_tensor("y", (D, S), y_dt, kind="ExternalOutput")

    with tile.TileContext(nc) as tc:
        with (
            tc.tile_pool(name="const", bufs=1) as cpool,
            tc.tile_pool(name="persist", bufs=1) as ppool,
            tc.tile_pool(name="state", bufs=1) as hpool,
            tc.tile_pool(name="work", bufs=3) as wpool,
        ):
            wih = cpool.tile([D, H4], wih_dt, name="wih")
            whh = cpool.tile([D, H4], whh_dt, name="whh")
            bc = cpool.tile([D, 4], F32, name="bc")
            ident = cpool.tile([D, D], id_dt, name="ident")
            xT = ppool.tile([D, XW], x_dt, name="xT")
            QW = XW // 4  # 516

            if warm_table:
                z16 = cpool.tile([D, 16], F32, name="z16")
                zs = cpool.tile([D, 16], F32, name="zs")
                nc.vector.memset(z16, 0.0)
                nc.scalar.activation(zs, z16, SIG)

            # DMA order matters: the first step-0 chunk needs xT q0 + wih +
            # bc; everything else can land later.
            if io_rows_in:
                # data lands at col 16 (32B-aligned: the DMA XBAR silently
                # corrupts transposed writes at unaligned SBUF offsets).
                # xT col c = x[c-16]; window of position s = cols s+1..s+16.
                nc.vector.memset(xT[:, 0:16], 0.0)
                nc.sync.dma_start_transpose(
                    xT[:, 16 : 16 + CH], x_d.ap()[0:CH, :]
                )
                nc.sync.dma_start(
                    out=wih,
                    in_=wih_d.ap() if x_bf16 else wih_d.ap().bitcast(F32R),
                )
                nc.sync.dma_start(out=bc, in_=b_d.ap())
                for q in range(1, 4):
                    nc.sync.dma_start_transpose(
                        xT[:, 16 + q * CH : 16 + (q + 1) * CH],
                        x_d.ap()[q * CH : (q + 1) * CH, :],
                    )
            else:
                nc.sync.dma_start(out=xT[:, 0:QW], in_=x_d.ap()[:, 0:QW])
                nc.sync.dma_start(
                    out=wih,
                    in_=wih_d.ap() if x_bf16 else wih_d.ap().bitcast(F32R),
                )
                nc.sync.dma_start(out=bc, in_=b_d.ap())
                for q in range(1, 4):
                    nc.sync.dma_start(
                        out=xT[:, q * QW : (q + 1) * QW],
                        in_=x_d.ap()[:, q * QW : (q + 1) * QW],
                    )
            nc.sync.dma_start(
                out=whh,
                in_=whh_d.ap() if whh_bf16 else whh_d.ap().bitcast(F32R),
            )
            nc.sync.dma_start(out=ident, in_=id_d.ap())
            xg_dt = BF16 if xg_bf16 else F32R
            xg = [ppool.tile([D, XW], xg_dt, name=f"xg{j}") for j in range(4)]

            h = [hpool.tile([D, CH], F32R, name=f"h{k}") for k in range(NCH)]
            c = [hpool.tile([D, CH], F32, name=f"c{k}") for k in range(NCH)]

            sig_insts = []
            hwr_insts = []
            segs = [(k * CH, CH) for k in range(4)] + [(4 * CH, XW - 4 * CH)]

            soff = 1 if io_rows_in else 0

            with tc.tile_pool(name="psum_g", bufs=2, space="PSUM") as pgp:

                def new_pg():
                    return pgp.tile([D, 4 * CH], F32, name="pg", tag="pg")

                def emit_xg_seg(si):
                    off, ln = segs[si]
                    pg = new_pg()
                    for j in range(4):
                        bank = pg[:, j * CH : j * CH + ln]
                        nc.tensor.matmul(
                            bank,
                            wih[:, j * D : (j + 1) * D],
                            xT[:, off : off + ln],
                            start=True,
                            stop=True,
                        )
                        nc.vector.tensor_scalar_add(
                            out=xg[j][:, off : off + ln],
                            in0=bank,
                            scalar1=bc[:, j : j + 1],
                        )

                def emit_cell_tail(w, k, s):
                    s_i = s[:, 0:CH]
                    s_f = s[:, CH : 2 * CH]
                    s_o = s[:, 2 * CH : 3 * CH]
                    s_g = s[:, 3 * CH : 4 * CH]
                    u = wpool.tile([D, CH], F32, name="u", tag="u")
                    nc.vector.scalar_tensor_tensor(u, s_g, -0.5, s_i, ADD, MUL)
                    if w > 0:
                        t2 = wpool.tile([D, CH], F32, name="t2", tag="t2")
                        nc.vector.tensor_tensor(t2, s_f, c[k], MUL)
                        nc.vector.scalar_tensor_tensor(c[k], u, 2.0, t2, MUL, ADD)
                    else:
                        nc.vector.tensor_scalar_mul(c[k], u, 2.0)
                    tc_t = wpool.tile([D, CH], F32, name="tc", tag="tc")
                    nc.scalar.activation(tc_t, c[k], TANH)
                    h_eng = nc.gpsimd if k in h_gpsimd else nc.vector
                    hwr_insts.append(
                        h_eng.tensor_tensor(h[k], tc_t, s_o, MUL)
                    )

                def emit_step0_chunk(k):
                    pg = new_pg()
                    s = wpool.tile([D, 4 * CH], F32, name="s", tag="s")
                    if step0_direct:
                        for j in range(4):
                            nc.tensor.matmul(
                                pg[:, j * CH : (j + 1) * CH],
                                wih[:, j * D : (j + 1) * D],
                                xT[:, k * CH + soff : (k + 1) * CH + soff],
                                start=True,
                                stop=True,
                            )
                        for j in range(4):
                            sig_insts.append(
                                nc.scalar.activation(
                                    s[:, j * CH : (j + 1) * CH],
                                    pg[:, j * CH : (j + 1) * CH],
                                    SIG,
                                    bias=bc[:, j : j + 1],
                                )
                            )
                    else:
                        for j in range(4):
                            nc.tensor.matmul(
                                pg[:, j * CH : (j + 1) * CH],
                                ident,
                                xg[j][:, k * CH : (k + 1) * CH],
                                start=True,
                                stop=True,
                            )
                        sig_insts.append(nc.scalar.activation(s, pg, SIG))
                    emit_cell_tail(0, k, s)

                def emit_step_chunk(w, k):
                    pg = new_pg()
                    if group_mm:
                        for j in range(4):
                            nc.tensor.matmul(
                                pg[:, j * CH : (j + 1) * CH],
                                whh[:, j * D : (j + 1) * D],
                                h[k],
                                start=True,
                                stop=True,
                            )
                        for j in range(4):
                            xsl = xg[j][:, k * CH + w + soff : k * CH + w + soff + CH]
                            nc.tensor.matmul(
                                pg[:, j * CH : (j + 1) * CH],
                                ident,
                                xsl,
                                start=False,
                                stop=True,
                                skip_group_check=True,
                            )
                    else:
                        for j in range(4):
                            bank = pg[:, j * CH : (j + 1) * CH]
                            xsl = xg[j][:, k * CH + w + soff : k * CH + w + soff + CH]
                            nc.tensor.matmul(
                                bank,
                                whh[:, j * D : (j + 1) * D],
                                h[k],
                                start=True,
                                stop=False,
                            )
                            nc.tensor.matmul(
                                bank, ident, xsl, start=False, stop=True
                            )
                    s = wpool.tile([D, 4 * CH], F32, name="s", tag="s")
                    sig_insts.append(nc.scalar.activation(s, pg, SIG))
                    emit_cell_tail(w, k, s)

                total_w = [wi for _ in range(reps) for wi in range(W)]
                for tok in early_order.split(","):
                    if tok.startswith("c"):
                        emit_step0_chunk(int(tok[1:]))
                    else:
                        emit_xg_seg(int(tok[1:]))
                for wi, w in enumerate(total_w):
                    if wi == 0:
                        continue
                    for k in range(NCH):
                        emit_step_chunk(w, k)

            # output: h chunks straight to DRAM (host transposes back), or
            # transposed on device (PE transpose per 128-block) for io_rows.
            if io_rows_out:
                identB = cpool.tile([D, D], BF16, name="identB")
                nc.gpsimd.tensor_copy(identB, ident.bitcast(F32))
                yrows = hpool.tile([D, S], BF16, name="yrows")
                with tc.tile_pool(name="psum_t", bufs=1, space="PSUM") as ptp:
                    for k in range(NCH):
                        yb = hpool.tile([D, CH], BF16, name=f"yb{k}")
                        nc.gpsimd.tensor_copy(yb, h[k].bitcast(F32))
                        pt = ptp.tile([D, CH // 2], F32, name="pt", tag="pt")
                        ptb = pt.bitcast(BF16)          # [D, CH] bf16 view
                        for j in range(4):
                            nc.tensor.transpose(
                                ptb[:, j * D : (j + 1) * D],
                                yb[:, j * D : (j + 1) * D],
                                identB,
                            )
                        nc.vector.tensor_copy(
                            yrows[:, k * CH : (k + 1) * CH], ptb
                        )
                        for j in range(4):
                            r0 = (4 * k + j) * D
                            nc.sync.dma_start(
                                out=y_d.ap()[r0 : r0 + D, :],
                                in_=yrows[:, r0 : r0 + D],
                            )
            elif y_bf16:
                yb = [hpool.tile([D, CH], BF16, name=f"yb{k}")
                      for k in range(NCH)]
                for k in range(NCH):
                    nc.gpsimd.tensor_copy(yb[k], h[k].bitcast(F32))
                    nc.sync.dma_start(
                        out=y_d.ap()[:, k * CH : (k + 1) * CH], in_=yb[k]
                    )
            else:
                for k in range(NCH):
                    nc.sync.dma_start(
                        out=y_d.ap()[:, k * CH : (k + 1) * CH],
                        in_=h[k].bitcast(F32),
                    )
    nc.compile()
    return nc


def prep_weights(w_ih, w_hh, b_ih, b_hh):
    """Gate-reorder to [i, f, o, g], fold both biases together, pre-scale the
    g-gate rows by 2 (its tanh is computed as 2*sigmoid(2g) - 1)."""
    w_ih = np.asarray(w_ih, np.float32)
    w_hh = np.asarray(w_hh, np.float32)
    b = np.asarray(b_ih, np.float32) + np.asarray(b_hh, np.float32)
    perm = np.r_[0:128, 128:256, 384:512, 256:384]
    sc = np.repeat(np.float32([1, 1, 1, 2]), D)
    wihT = np.ascontiguousarray((w_ih[perm] * sc[:, None]).T, np.float32)
    whhT = np.ascontiguousarray((w_hh[perm] * sc[:, None]).T, np.float32)
    bcols = np.ascontiguousarray((b[perm] * sc).reshape(4, D).T, np.float32)
    return wihT, whhT, bcols


def prep_x(x):
    """(B, S, D) -> per-core padded transposed xT (B, D, PAD+S+1)."""
    x = np.asarray(x, np.float32)
    xt = np.zeros((B, D, XW), np.float32)
    xt[:, :, PAD : PAD + S] = x.transpose(0, 2, 1)
    return xt


class _Runner:
    """Process-lifetime cache: compiled NEFF + jitted 8-core executable +
    device-resident weights.  Per call: upload x, execute, fetch y."""

    BUILD_KWARGS = {"io_rows": True}

    def __init__(self, build_kwargs=None):
        import jax
        from jax.sharding import Mesh, PartitionSpec, NamedSharding
        from jax.experimental.shard_map import shard_map
        from concourse import bass2jax as b2j
        import ml_dtypes

        self._jax = jax
        self._bf16 = ml_dtypes.bfloat16
        b2j.install_neuronx_cc_hook()
        if build_kwargs is None:
            build_kwargs = dict(self.BUILD_KWARGS)
        self.build_kwargs = build_kwargs
        self._io_rows = bool(build_kwargs.get("io_rows", False))
        self._x_bf16 = self._io_rows or bool(build_kwargs.get("x_bf16", False))
        self._y_bf16 = self._io_rows or bool(build_kwargs.get("y_bf16", False))
        self.nc = build_nc(**build_kwargs)
        nc = self.nc
        partition_name = (
            nc.partition_id_tensor.name if nc.partition_id_tensor else None
        )
        in_names, out_names, out_avals, zero_outs = [], [], [], []
        for alloc in nc.m.functions[0].allocations:
            if not isinstance(alloc, mybir.MemoryLocationSet):
                continue
            name = alloc.memorylocations[0].name
            if alloc.kind == "ExternalInput":
                if name != partition_name:
                    in_names.append(name)
            elif alloc.kind == "ExternalOutput":
                shape = tuple(alloc.tensor_shape)
                dtype = mybir.dt.np(alloc.dtype)
                out_names.append(name)
                out_avals.append(jax.core.ShapedArray(shape, dtype))
                zero_outs.append(np.zeros(shape, dtype))
        self.in_names = in_names
        self.out_names = out_names
        all_in_names = list(in_names) + out_names
        if partition_name is not None:
            all_in_names.append(partition_name)

        def _body(*args):
            operands = list(args)
            if partition_name is not None:
                operands.append(b2j.partition_id_tensor())
            outs = b2j._bass_exec_p.bind(
                *operands,
                out_avals=tuple(out_avals),
                in_names=tuple(all_in_names),
                out_names=tuple(out_names),
                lowering_input_output_aliases=(),
                sim_require_finite=True,
                sim_require_nnan=True,
                nc=nc,
            )
            return tuple(outs)

        devices = jax.devices()[:B]
        mesh = Mesh(np.asarray(devices), ("core",))
        n_params = len(in_names)
        n_outs = len(out_names)
        self.sharded = jax.jit(
            shard_map(
                _body,
                mesh=mesh,
                in_specs=(PartitionSpec("core"),) * (n_params + n_outs),
                out_specs=(PartitionSpec("core"),) * n_outs,
                check_rep=False,
            ),
            keep_unused=True,
        )
        self.sharding = NamedSharding(mesh, PartitionSpec("core"))
        self.dev_zeros = [
            jax.device_put(
                np.zeros((B * z.shape[0], *z.shape[1:]), z.dtype), self.sharding
            )
            for z in zero_outs
        ]
        self._wkey = None
        self._dev_w = None
        # reusable host staging buffer for the concatenated x
        xdt = self._bf16 if self._x_bf16 else np.float32
        if self._io_rows:
            self._xbuf = np.zeros((B * S, D), xdt)
        else:
            self._xbuf = np.zeros((B * D, XW), xdt)

    def _stage_weights(self, w_ih, w_hh, b_ih, b_hh):
        w_ih = np.asarray(w_ih, np.float32)
        w_hh = np.asarray(w_hh, np.float32)
        b_ih = np.asarray(b_ih, np.float32)
        b_hh = np.asarray(b_hh, np.float32)
        key = (
            w_ih.tobytes(), w_hh.tobytes(), b_ih.tobytes(), b_hh.tobytes(),
        )
        if self._wkey == key:
            return
        wihT, whhT, bcols = prep_weights(w_ih, w_hh, b_ih, b_hh)
        if self._x_bf16:
            wihT = wihT.astype(self._bf16)
        ident = np.eye(D, dtype=np.float32)
        per_name = {"wihT": wihT, "whhT": whhT, "bcols": bcols, "ident": ident}
        self._dev_w = {
            nm: self._jax.device_put(
                np.concatenate([arr] * B, 0), self.sharding
            )
            for nm, arr in per_name.items()
        }
        self._wkey = key

    def __call__(self, x, w_ih, w_hh, b_ih, b_hh):
        self._stage_weights(w_ih, w_hh, b_ih, b_hh)
        x = np.asarray(x, np.float32)
        xb = self._xbuf
        if self._io_rows:
            np.copyto(xb.reshape(B, S, D), x, casting="unsafe")
            xkey = "xR"
        else:
            for bidx in range(B):
                xb[bidx * D : (bidx + 1) * D, PAD : PAD + S] = x[bidx].T
            xkey = "xT"
        args = [
            xb if nm == xkey else self._dev_w[nm] for nm in self.in_names
        ]
        out = self.sharded(*args, *self.dev_zeros)
        y = np.asarray(out[0])
        if self._io_rows:                           # (B*S, D) bf16
            return y.reshape(B, S, D).astype(np.float32)
        res = np.empty((B, S, D), np.float32)       # (B*D, S) f32 or bf16
        yr = y.reshape(B, D, S)
        for bidx in range(B):
            res[bidx] = yr[bidx].T                  # casts bf16 -> f32
        return res


_RUNNER = None


def _get_runner():
    global _RUNNER
    if _RUNNER is None:
        _RUNNER = _Runner()
    return _RUNNER


def kernel(x, w_ih, w_hh, b_ih, b_hh, window_size):
    assert int(window_size) == W, window_size
    return _get_runner()(x, w_ih, w_hh, b_ih, b_hh)


# ---- legacy helpers kept for test harnesses ---------------------------------

_NC_CACHE = {}


def _get_nc(mm_dtype=F32R):
    key = str(mm_dtype)
    if key not in _NC_CACHE:
        _NC_CACHE[key] = build_nc(mm_dtype)
    return _NC_CACHE[key]


def run(x, w_ih, w_hh, b_ih, b_hh, trace=False, mm_dtype=F32R, **spmd_kwargs):
    from concourse.bass_utils import run_bass_kernel_spmd

    x = np.asarray(x, np.float32)
    assert x.shape == (B, S, D), x.shape
    wihT, whhT, bcols = prep_weights(w_ih, w_hh, b_ih, b_hh)
    xt = prep_x(x)
    nc = _get_nc(mm_dtype)
    ident = np.eye(D, dtype=np.float32)
    in_maps = [
        {"xT": xt[cid], "wihT": wihT, "whhT": whhT, "bcols": bcols,
         "ident": ident}
        for cid in range(B)
    ]
    res = run_bass_kernel_spmd(
        nc, in_maps, core_ids=list(range(B)), trace=trace, **spmd_kwargs
    )
    out = np.ascontiguousarray(
        np.stack([res.results[cid]["y"] for cid in range(B)], 0).transpose(
            0, 2, 1
        )
    )
    return out, res
